# revision 6
# baseline (speedup 1.0000x reference)
"""Multi-head attention (B=16, S=1024, D=768, H=12) on 8 TRN2 NeuronCores.

Strategy: pure data parallelism — batch 16 is split 2-per-core; weights are
replicated. Each core runs an identical Bass/Tile program on its own x shard,
so no collectives are needed. Host-side marshaling pre-transposes x and the
weights into the d-major layouts the PE array contracts over.

Per-core program (b in 0..1, head-pairs hp in 0..5):
  - v  = x @ W_v^T           natural [t, e] layout, stored head-interleaved
                             with a ones column -> PV lhsT [k, 64+1] per head
  - qT2/kT2 [128, S]         two heads stacked on partitions (d-major)
  - scoresT[k,q] = k q^T     row-packed per head via tile_position (K=64)
  - exp on ACT (scale=1/8) -> f32r SBUF tile
  - PV: out[dh+1, q] += v_ext.T @ exp   (row 64 accumulates the softmax denom)
  - 1/denom on DVE (fast approx); gpsimd partition_broadcast across 64
    partitions; DVE mult normalizes into attn_outT [d, t]
  - y = attn_outT.T @ W_out^T + b_out  (bias folded in as a K=1 matmul)

All matmuls run as float32r (1 cycle/row at free-dim >=256, ~1.5e-4 rel err).
PSUM pools: scores [128,1024]x2 (4 banks) | gen [128,512]x1 (1 bank) |
oacc [65,512]x3 (3 banks) — gen work no longer serializes against attention.
"""
import numpy as np
import concourse.bacc as bacc
import concourse.tile as tile
from concourse import mybir
from concourse.bass_utils import run_bass_kernel_spmd

FP32 = mybir.dt.float32
FP32R = mybir.dt.float32r
EXP = mybir.ActivationFunctionType.Exp

B, S, D, H = 2, 1024, 768, 12       # per-core batch of 2
HP = H // 2                          # head pairs
DT = D // 128                        # d tiles (6)
KT = S // 128                        # k tiles (8)
QC = S // 512                        # q chunks (2)
TT = S // 128                        # t tiles per batch (8)
N_CORES = 8

_CACHE = {}


def build_nc():
    nc = bacc.Bacc(trn_type="TRN2")
    xT = nc.dram_tensor("xT", [D, B * S], FP32R, kind="ExternalInput")
    wqkvT = nc.dram_tensor("wqkvT", [D, 3 * D], FP32R, kind="ExternalInput")
    woutT = nc.dram_tensor("woutT", [D, D], FP32R, kind="ExternalInput")
    bout = nc.dram_tensor("bout", [1, D], FP32R, kind="ExternalInput")
    ones_d = nc.dram_tensor("ones_d", [128, 128], FP32R, kind="ExternalInput")
    y = nc.dram_tensor("y", [B * S, D], FP32, kind="ExternalOutput")

    with tile.TileContext(nc) as tc:
        with (
            tc.tile_pool(name="wq", bufs=1) as p_wq,
            tc.tile_pool(name="wo", bufs=1) as p_wo,
            tc.tile_pool(name="cst", bufs=1) as p_cst,
            tc.tile_pool(name="xt", bufs=1) as p_xt,
            tc.tile_pool(name="vv", bufs=1) as p_v,
            tc.tile_pool(name="ao", bufs=1) as p_ao,
            tc.tile_pool(name="qk", bufs=4) as p_qk,
            tc.tile_pool(name="exp", bufs=3) as p_exp,
            tc.tile_pool(name="rr", bufs=2) as p_r,
            tc.tile_pool(name="yy", bufs=2) as p_y,
            tc.tile_pool(name="rb", bufs=2) as p_rb,
            tc.tile_pool(name="sc", bufs=2, space="PSUM") as p_sc,
            tc.tile_pool(name="gen", bufs=1, space="PSUM") as p_gen,
            tc.tile_pool(name="oacc", bufs=3, space="PSUM") as p_oacc,
        ):
            wq = p_wq.tile([128, DT, 3 * D], FP32R)
            wo = p_wo.tile([128, DT, D], FP32R)
            bo = p_cst.tile([1, D], FP32R)
            ones = p_cst.tile([1, 128], FP32R)
            nc.sync.dma_start(bo[:], bout[:])
            nc.sync.dma_start(ones[:], ones_d[0:1, :])
            for j in range(DT):
                nc.sync.dma_start(wq[:, j, :], wqkvT[128 * j:128 * (j + 1), :])
                nc.sync.dma_start(wo[:, j, :], woutT[128 * j:128 * (j + 1), :])

            def qk_gen(qp_pool, xt, part, hp, qc):
                """One [128,512] psum group: q or k for head pair hp, chunk qc."""
                qp = qp_pool.tile([128, 512], FP32, tag="gen")
                for j in range(DT):
                    nc.tensor.matmul(
                        qp[:, :],
                        wq[:, j, part * D + 128 * hp:part * D + 128 * (hp + 1)],
                        xt[:, j, qc * 512:(qc + 1) * 512],
                        start=(j == 0), stop=(j == DT - 1),
                    )
                return qp

            for b in range(B):
                xt = p_xt.tile([128, DT, S], FP32R, tag="xt")
                for j in range(DT):
                    nc.sync.dma_start(
                        xt[:, j, :], xT[128 * j:128 * (j + 1), b * S:(b + 1) * S]
                    )

                # ---- v generation: v[t, e] for all 12 heads, head-interleaved
                # [128, kt, h, 65] with col 64 = 1.0 (softmax denom rider)
                v = p_v.tile([128, KT, H, 65], FP32R, tag="vv")
                nc.sync.dma_start(
                    v[:, :, :, 64],
                    ones_d[:, 0:KT * H].rearrange("p (k h) -> p k h", k=KT),
                )
                for tt in range(TT):
                    for c, (h0, nh) in enumerate(((0, 8), (8, 4))):
                        vp = p_gen.tile([128, 512], FP32, tag="gen")
                        cw = nh * 64
                        for j in range(DT):
                            nc.tensor.matmul(
                                vp[:, 0:cw],
                                xt[:, j, tt * 128:(tt + 1) * 128],
                                wq[:, j, 2 * D + h0 * 64:2 * D + h0 * 64 + cw],
                                start=(j == 0), stop=(j == DT - 1),
                            )
                        nc.vector.tensor_copy(
                            v[:, tt, h0:h0 + nh, 0:64],
                            vp[:, 0:cw].rearrange("p (h c) -> p h c", h=nh),
                        )

                # attn_outT [d, t] accumulator for this batch; each hp writes
                # a disjoint d-tile band
                ao = p_ao.tile([128, DT, S], FP32R, tag="ao")

                for hp in range(HP):
                    # ---- q/k generation for this head pair (2 heads stacked)
                    qkt = []
                    for part in range(2):  # 0 = q, 1 = k
                        sq = p_qk.tile([128, S], FP32R, tag="qk")
                        for qc in range(QC):
                            qp = qk_gen(p_gen, xt, part, hp, qc)
                            nc.vector.tensor_copy(
                                sq[:, qc * 512:(qc + 1) * 512], qp[:, :]
                            )
                        qkt.append(sq)
                    qT2, kT2 = qkt

                    for qc in range(QC):
                        oA = p_oacc.tile([65, 512], FP32, tag="oacc")
                        oB = p_oacc.tile([65, 512], FP32, tag="oacc")
                        for kt in range(KT):
                            sc = p_sc.tile([128, 1024], FP32, tag="sc")
                            nc.tensor.matmul(
                                sc[:, 0:512],
                                kT2[0:64, kt * 128:(kt + 1) * 128],
                                qT2[0:64, qc * 512:(qc + 1) * 512],
                                start=True, stop=True, tile_position=(0, 0),
                            )
                            nc.tensor.matmul(
                                sc[:, 512:1024],
                                kT2[64:128, kt * 128:(kt + 1) * 128],
                                qT2[64:128, qc * 512:(qc + 1) * 512],
                                start=True, stop=True, tile_position=(64, 0),
                            )
                            ex = p_exp.tile([128, 1024], FP32R, tag="exp")
                            nc.scalar.activation(ex[:], sc[:], EXP, scale=0.125)
                            nc.tensor.matmul(
                                oA[:], v[:, kt, 2 * hp, :], ex[:, 0:512],
                                start=(kt == 0), stop=(kt == KT - 1),
                            )
                            nc.tensor.matmul(
                                oB[:], v[:, kt, 2 * hp + 1, :], ex[:, 512:1024],
                                start=(kt == 0), stop=(kt == KT - 1),
                            )
                        # normalize: fast recip of denom row, gpsimd broadcast
                        # across partitions, DVE mult into attn_outT (head B
                        # shifts partitions on the write)
                        for o, head in ((oA, 0), (oB, 1)):
                            r = p_r.tile([1, 512], FP32, tag="rr")
                            nc.vector.reciprocal(r[:], o[64:65, :])
                            rb = p_rb.tile([64, 512], FP32, tag="rb")
                            nc.gpsimd.partition_broadcast(rb[:], r[:])
                            nc.vector.tensor_mul(
                                ao[64 * head:64 * (head + 1), hp,
                                   qc * 512:(qc + 1) * 512],
                                o[0:64, :], rb[:],
                            )

                # ---- output projection for this batch
                for tt in range(TT):
                    ys = p_y.tile([128, D], FP32, tag="yy")
                    for c0, cw in ((0, 512), (512, 256)):
                        yp = p_gen.tile([128, 512], FP32, tag="gen")
                        nc.tensor.matmul(
                            yp[:, 0:cw], ones[:], bo[:, c0:c0 + cw],
                            start=True, stop=False,
                        )
                        for j in range(DT):
                            nc.tensor.matmul(
                                yp[:, 0:cw],
                                ao[:, j, tt * 128:(tt + 1) * 128],
                                wo[:, j, c0:c0 + cw],
                                start=False, stop=(j == DT - 1),
                            )
                        nc.vector.tensor_copy(ys[:, c0:c0 + cw], yp[:, 0:cw])
                    nc.sync.dma_start(
                        y[b * S + tt * 128:b * S + (tt + 1) * 128, :], ys[:]
                    )
    nc.finalize()
    return nc


def _marshal(x, W_qkv, W_out, b_out):
    wqkvT = np.ascontiguousarray(W_qkv.T).astype(np.float32)
    woutT = np.ascontiguousarray(W_out.T).astype(np.float32)
    bo = np.ascontiguousarray(b_out.reshape(1, D)).astype(np.float32)
    ones = np.ones((128, 128), dtype=np.float32)
    in_maps = []
    for c in range(N_CORES):
        xc = np.ascontiguousarray(
            np.asarray(x)[B * c:B * (c + 1)].reshape(B * S, D).T
        ).astype(np.float32)
        in_maps.append({
            "xT": xc, "wqkvT": wqkvT, "woutT": woutT, "bout": bo,
            "ones_d": ones,
        })
    return in_maps


def run(x, W_qkv, W_out, b_out, trace=False, **spmd_kwargs):
    if "nc" not in _CACHE:
        _CACHE["nc"] = build_nc()
    nc = _CACHE["nc"]
    in_maps = _marshal(x, W_qkv, W_out, b_out)
    res = run_bass_kernel_spmd(
        nc, in_maps, core_ids=list(range(N_CORES)), trace=trace, **spmd_kwargs
    )
    out = np.stack([res.results[c]["y"] for c in range(N_CORES)], axis=0)
    out = out.reshape(N_CORES * B, S, D)
    return out, res


def kernel(x, W_qkv, W_out, b_out):
    out, _ = run(x, W_qkv, W_out, b_out)
    return out


# revision 9
# speedup vs baseline: 1.5217x; 1.5217x over previous
"""Multi-head attention (B=16, S=1024, D=768, H=12) on 8 TRN2 NeuronCores.

Strategy: pure data parallelism — batch 16 is split 2-per-core; weights are
replicated. Each core runs an identical Bass/Tile program on its own x shard,
so no collectives are needed. Host-side marshaling pre-transposes x and the
weights into the d-major layouts the PE array contracts over.

Per-core program (b in 0..1, head-pairs hp in 0..5):
  - v  = x @ W_v^T           natural [t, e] layout, stored head-interleaved
                             with a ones column -> PV lhsT [k, 64+1] per head
  - qT2/kT2 [128, S]         two heads stacked on partitions (d-major)
  - scoresT[k,q] = k q^T     row-packed per head via tile_position (K=64)
  - exp on ACT (scale=1/8) -> f32r SBUF tile
  - PV: out[dh+1, q] += v_ext.T @ exp   (row 64 accumulates the softmax denom)
  - normalize: one [65,512] copy to SBUF per accumulator (frees PSUM fast),
    denom rows staged into a [4,512] tile, ONE batched DVE reciprocal per
    unit (iterative-divide cost is per-lane, so stacking rows is ~free),
    gpsimd partition_broadcast, DVE mult -> attn_outT [d, t] (bf16)
  - y = attn_outT.T @ W_out^T + b_out  (bf16 matmuls; bias as K=1 matmul)

Attention/QKV matmuls run as float32r (1 cycle/row at free-dim >=256,
~1.5e-4 rel err); the final projection runs bf16 (~2e-3) to halve its
SBUF footprint and weight-load cost.
"""
import ml_dtypes
import numpy as np
import concourse.bacc as bacc
import concourse.tile as tile
from concourse import mybir
from concourse.bass_utils import run_bass_kernel_spmd

FP32 = mybir.dt.float32
FP32R = mybir.dt.float32r
BF16 = mybir.dt.bfloat16
EXP = mybir.ActivationFunctionType.Exp

B, S, D, H = 2, 1024, 768, 12       # per-core batch of 2
HP = H // 2                          # head pairs
DT = D // 128                        # d tiles (6)
KT = S // 128                        # k tiles (8)
QC = S // 512                        # q chunks (2)
TT = S // 128                        # t tiles per batch (8)
N_CORES = 8

_CACHE = {}


def build_nc():
    nc = bacc.Bacc(trn_type="TRN2")
    xT = nc.dram_tensor("xT", [D, B * S], FP32R, kind="ExternalInput")
    wqkvT = nc.dram_tensor("wqkvT", [D, 3 * D], FP32R, kind="ExternalInput")
    woutT = nc.dram_tensor("woutT", [D, D], BF16, kind="ExternalInput")
    bout = nc.dram_tensor("bout", [1, D], BF16, kind="ExternalInput")
    ones_d = nc.dram_tensor("ones_d", [128, 128], FP32R, kind="ExternalInput")
    y = nc.dram_tensor("y", [B * S, D], FP32, kind="ExternalOutput")

    with tile.TileContext(nc) as tc:
        with (
            tc.tile_pool(name="wq", bufs=1) as p_wq,
            tc.tile_pool(name="wo", bufs=1) as p_wo,
            tc.tile_pool(name="cst", bufs=1) as p_cst,
            tc.tile_pool(name="xt", bufs=1) as p_xt,
            tc.tile_pool(name="vv", bufs=1) as p_v,
            tc.tile_pool(name="ao", bufs=1) as p_ao,
            tc.tile_pool(name="qk", bufs=4) as p_qk,
            tc.tile_pool(name="exp", bufs=3) as p_exp,
            tc.tile_pool(name="oc", bufs=6) as p_oc,
            tc.tile_pool(name="dn", bufs=2) as p_dn,
            tc.tile_pool(name="yy", bufs=2) as p_y,
            tc.tile_pool(name="rb", bufs=2) as p_rb,
            tc.tile_pool(name="r0", bufs=2) as p_r0,
            tc.tile_pool(name="sc", bufs=2, space="PSUM") as p_sc,
            tc.tile_pool(name="gen", bufs=2, space="PSUM") as p_gen,
            tc.tile_pool(name="oacc", bufs=2, space="PSUM") as p_oacc,
        ):
            wq = p_wq.tile([128, DT, 3 * D], FP32R)
            wo = p_wo.tile([128, DT, D], BF16)
            bo = p_cst.tile([1, D], BF16)
            ones = p_cst.tile([1, 128], FP32R)
            ones_bf = p_cst.tile([1, 128], BF16)
            nc.sync.dma_start(bo[:], bout[:])
            nc.sync.dma_start(ones[:], ones_d[0:1, :])
            nc.vector.tensor_copy(ones_bf[:], ones[:].bitcast(FP32))
            for j in range(DT):
                nc.sync.dma_start(wq[:, j, :], wqkvT[128 * j:128 * (j + 1), :])
                nc.sync.dma_start(wo[:, j, :], woutT[128 * j:128 * (j + 1), :])

            def qk_gen(xt, part, hp, qc):
                """One [128,512] psum group: q or k for head pair hp, chunk qc."""
                qp = p_gen.tile([128, 512], FP32, tag="gen")
                for j in range(DT):
                    nc.tensor.matmul(
                        qp[:, :],
                        wq[:, j, part * D + 128 * hp:part * D + 128 * (hp + 1)],
                        xt[:, j, qc * 512:(qc + 1) * 512],
                        start=(j == 0), stop=(j == DT - 1),
                    )
                return qp

            for b in range(B):
                xt = p_xt.tile([128, DT, S], FP32R, tag="xt")
                for j in range(DT):
                    nc.sync.dma_start(
                        xt[:, j, :], xT[128 * j:128 * (j + 1), b * S:(b + 1) * S]
                    )

                # ---- v generation: v[t, e] for all 12 heads, head-interleaved
                # [128, kt, h, 65] with col 64 = 1.0 (softmax denom rider)
                v = p_v.tile([128, KT, H, 65], FP32R, tag="vv")
                nc.sync.dma_start(
                    v[:, :, :, 64],
                    ones_d[:, 0:KT * H].rearrange("p (k h) -> p k h", k=KT),
                )
                for tt in range(TT):
                    for h0, nh in ((0, 8), (8, 4)):
                        vp = p_gen.tile([128, 512], FP32, tag="gen")
                        cw = nh * 64
                        for j in range(DT):
                            nc.tensor.matmul(
                                vp[:, 0:cw],
                                xt[:, j, tt * 128:(tt + 1) * 128],
                                wq[:, j, 2 * D + h0 * 64:2 * D + h0 * 64 + cw],
                                start=(j == 0), stop=(j == DT - 1),
                            )
                        nc.vector.tensor_copy(
                            v[:, tt, h0:h0 + nh, 0:64],
                            vp[:, 0:cw].rearrange("p (h c) -> p h c", h=nh),
                        )

                # attn_outT [d, t] accumulator for this batch; each hp writes
                # a disjoint d-tile band
                ao = p_ao.tile([128, DT, S], BF16, tag="ao")

                for hp in range(HP):
                    # ---- q/k generation for this head pair (2 heads stacked)
                    qkt = []
                    for part in range(2):  # 0 = q, 1 = k
                        sq = p_qk.tile([128, S], FP32R, tag="qk")
                        for qc in range(QC):
                            qp = qk_gen(xt, part, hp, qc)
                            nc.vector.tensor_copy(
                                sq[:, qc * 512:(qc + 1) * 512], qp[:, :]
                            )
                        qkt.append(sq)
                    qT2, kT2 = qkt

                    ocs = {}
                    # denom rows live at partitions 0/32/64/96 (32-aligned
                    # base-partition requirement); padding rows are memset to
                    # 1.0 so the batched reciprocal stays finite
                    dn = p_dn.tile([128, 512], FP32, tag="dn")
                    nc.vector.memset(dn[:], 1.0)
                    for qc in range(QC):
                        oA = p_oacc.tile([65, 512], FP32, tag="oacc")
                        oB = p_oacc.tile([65, 512], FP32, tag="oacc")
                        for kt in range(KT):
                            sc = p_sc.tile([128, 1024], FP32, tag="sc")
                            nc.tensor.matmul(
                                sc[:, 0:512],
                                kT2[0:64, kt * 128:(kt + 1) * 128],
                                qT2[0:64, qc * 512:(qc + 1) * 512],
                                start=True, stop=True, tile_position=(0, 0),
                            )
                            nc.tensor.matmul(
                                sc[:, 512:1024],
                                kT2[64:128, kt * 128:(kt + 1) * 128],
                                qT2[64:128, qc * 512:(qc + 1) * 512],
                                start=True, stop=True, tile_position=(64, 0),
                            )
                            ex = p_exp.tile([128, 1024], FP32R, tag="exp")
                            nc.scalar.activation(ex[:], sc[:], EXP, scale=0.125)
                            nc.tensor.matmul(
                                oA[:], v[:, kt, 2 * hp, :], ex[:, 0:512],
                                start=(kt == 0), stop=(kt == KT - 1),
                            )
                            nc.tensor.matmul(
                                oB[:], v[:, kt, 2 * hp + 1, :], ex[:, 512:1024],
                                start=(kt == 0), stop=(kt == KT - 1),
                            )
                        # one copy frees the PSUM accumulator; stage the denom
                        # row into dn for the batched per-unit reciprocal
                        for o, head in ((oA, 0), (oB, 1)):
                            oc = p_oc.tile([65, 512], FP32, tag="oc")
                            nc.vector.tensor_copy(oc[:], o[:])
                            r_idx = 32 * (2 * qc + head)
                            nc.vector.tensor_copy(
                                dn[r_idx:r_idx + 1, :], oc[64:65, :]
                            )
                            ocs[(qc, head)] = oc
                    # batched reciprocal of the unit's 4 denom rows
                    dnr = p_dn.tile([128, 512], FP32, tag="dnr")
                    nc.vector.reciprocal(dnr[:], dn[:])
                    for qc in range(QC):
                        for head in range(2):
                            r_idx = 32 * (2 * qc + head)
                            # partition_broadcast only honors base-partition-0
                            # inputs on HW; shift the row down first
                            r0 = p_r0.tile([1, 512], FP32, tag="r0")
                            nc.vector.tensor_copy(r0[:], dnr[r_idx:r_idx + 1, :])
                            rb = p_rb.tile([64, 512], FP32, tag="rb")
                            nc.gpsimd.partition_broadcast(rb[:], r0[:])
                            nc.vector.tensor_mul(
                                ao[64 * head:64 * (head + 1), hp,
                                   qc * 512:(qc + 1) * 512],
                                ocs[(qc, head)][0:64, :], rb[:],
                            )

                # ---- output projection for this batch (bf16)
                for tt in range(TT):
                    ys = p_y.tile([128, D], FP32, tag="yy")
                    for c0, cw in ((0, 512), (512, 256)):
                        yp = p_gen.tile([128, 512], FP32, tag="gen")
                        nc.tensor.matmul(
                            yp[:, 0:cw], ones_bf[:], bo[:, c0:c0 + cw],
                            start=True, stop=False,
                        )
                        for j in range(DT):
                            nc.tensor.matmul(
                                yp[:, 0:cw],
                                ao[:, j, tt * 128:(tt + 1) * 128],
                                wo[:, j, c0:c0 + cw],
                                start=False, stop=(j == DT - 1),
                            )
                        nc.vector.tensor_copy(ys[:, c0:c0 + cw], yp[:, 0:cw])
                    nc.sync.dma_start(
                        y[b * S + tt * 128:b * S + (tt + 1) * 128, :], ys[:]
                    )
    nc.finalize()
    return nc


def _marshal(x, W_qkv, W_out, b_out):
    wqkvT = np.ascontiguousarray(W_qkv.T).astype(np.float32)
    woutT = np.ascontiguousarray(W_out.T).astype(ml_dtypes.bfloat16)
    bo = np.ascontiguousarray(b_out.reshape(1, D)).astype(ml_dtypes.bfloat16)
    ones = np.ones((128, 128), dtype=np.float32)
    in_maps = []
    for c in range(N_CORES):
        xc = np.ascontiguousarray(
            np.asarray(x)[B * c:B * (c + 1)].reshape(B * S, D).T
        ).astype(np.float32)
        in_maps.append({
            "xT": xc, "wqkvT": wqkvT, "woutT": woutT, "bout": bo,
            "ones_d": ones,
        })
    return in_maps


def run(x, W_qkv, W_out, b_out, trace=False, **spmd_kwargs):
    if "nc" not in _CACHE:
        _CACHE["nc"] = build_nc()
    nc = _CACHE["nc"]
    in_maps = _marshal(x, W_qkv, W_out, b_out)
    res = run_bass_kernel_spmd(
        nc, in_maps, core_ids=list(range(N_CORES)), trace=trace, **spmd_kwargs
    )
    out = np.stack([res.results[c]["y"] for c in range(N_CORES)], axis=0)
    out = out.reshape(N_CORES * B, S, D)
    return out, res


def kernel(x, W_qkv, W_out, b_out):
    out, _ = run(x, W_qkv, W_out, b_out)
    return out
